# revision 16
# baseline (speedup 1.0000x reference)
"""Trainium2 Bass kernel for nn_AttentionBlock (B=8, C=1024, L=1024, H=16, G=32).

Data-parallel over batch: one sample per NeuronCore, no collectives.
Per-core structure (v3 — fp8 DoubleRow for qkv/v/mm2/proj, bf16 mm1):

  1. GroupNorm, pipelined per 128-channel tile (each tile holds exactly 4
     groups, so stats never cross tiles): DVE row-sum + ACT Square-accum
     per tile feeding tiny per-tile selector matmuls into one [4, 8, 2]
     stats bank; the Ln/Exp rsqrt and the scale/bias algebra run ONCE,
     batched over all tiles (avoids per-tile ACT table swaps between the
     Square set and the Ln/Exp set).  Apply writes fp8 DoubleRow-layout
     tiles xq[kc2] = [128, 2, L] (channel c = 256*kc2 + 128*i + p) plus
     f32 residual tiles.
  2. q/k projections: fp8 DoubleRow matmuls (contraction 256/step),
     bias added on the PSUM->SBUF copy, output bf16 pair-packed [128, L]
     (head 2j on partitions 0:64, 2j+1 on 64:128).  v^T is produced
     directly in [L, (h, 65)] fp8 layout by swapping operands (lhsT =
     xq l-slice); a constant ones column per head feeds the softmax
     denominator through mm2.
  3. Attention per (pair, tcn): loop sc: two bf16 mm1s (the heads run
     concurrently on PE row groups 0/64) into a double-buffered
     [128, 2, 512] PSUM tile; one ACT exp(z/8 - 2ln2) -> fp8 slice of a
     [128, (sch, par), 512] tile (e4m3 max 240, max ex ~101; the shift
     cancels in normalization); per sc-pair two fp8 DoubleRow mm2s
     (contraction 256 = two s-chunks) accumulate [a_raw; S].  Epilogue:
     copy S row + a_raw to SBUF (fast bank release; also
     reciprocal_approx_fast misreads PSUM on HW), reciprocal, gpsimd
     partition-broadcast, one DVE multiply into the fp8 a-tile.  The
     next pair's q/k projection interleaves through a generator.
  4. proj: fp8 DoubleRow + (bias_eff + x_norm) residual epilogue, DMA out.

Weights are repacked host-side into DoubleRow lhsT layouts; v-bias is
folded into the proj bias (softmax rows sum to 1).
"""

import numpy as np
import ml_dtypes

import concourse.bass as bass
import concourse.bacc as bacc
import concourse.tile as tile
from concourse import mybir
from concourse.bass_utils import run_bass_kernel_spmd

F32 = mybir.dt.float32
BF16 = mybir.dt.bfloat16
FP8 = mybir.dt.float8e4
DR = mybir.MatmulPerfMode.DoubleRow
NPFP8 = ml_dtypes.float8_e4m3  # matches mybir.dt.float8e4 (IEEE, max 240)

B, C, L, H = 8, 1024, 1024, 16
GROUPS = 32
CH = C // H          # 64 per-head channels
EPS = 1e-5
NT = C // 128        # 8 channel tiles
KC2 = 4              # DoubleRow contraction steps (256 channels each)
LT = L // 512        # 2 free-dim chunks of 512
PAIRS = H // 2       # 8 head pairs
EXP_BIAS = -1.3862944  # -2*ln2: max exp(z/8-2ln2) ~ 101 < 240 (e4m3 max)


def declare_params(nc):
    p = {}
    p["x"] = nc.declare_dram_parameter("x", [C, L], F32, isOutput=False)
    # DoubleRow lhsT packs, partition-major: [128, (otile, kc2, 2, out)]
    p["q_w8"] = nc.declare_dram_parameter("q_w8", [128, PAIRS * KC2 * 256],
                                          FP8, isOutput=False)
    p["k_w8"] = nc.declare_dram_parameter("k_w8", [128, PAIRS * KC2 * 256],
                                          FP8, isOutput=False)
    p["v_w8"] = nc.declare_dram_parameter("v_w8", [128, 2 * KC2 * 1024],
                                          FP8, isOutput=False)
    p["p_w8"] = nc.declare_dram_parameter("p_w8", [128, NT * KC2 * 256],
                                          FP8, isOutput=False)
    p["q_b"] = nc.declare_dram_parameter("q_b", [128, NT], F32, isOutput=False)
    p["k_b"] = nc.declare_dram_parameter("k_b", [128, NT], F32, isOutput=False)
    p["proj_beff"] = nc.declare_dram_parameter("proj_beff", [128, NT], F32,
                                               isOutput=False)
    p["norm_w_c"] = nc.declare_dram_parameter("norm_w_c", [128, NT], F32,
                                              isOutput=False)
    p["norm_b_c"] = nc.declare_dram_parameter("norm_b_c", [128, NT], F32,
                                              isOutput=False)
    p["A_grp"] = nc.declare_dram_parameter("A_grp", [128, 4], F32,
                                           isOutput=False)
    p["A2T"] = nc.declare_dram_parameter("A2T", [4, 128], F32, isOutput=False)
    p["out"] = nc.declare_dram_parameter("out", [C, L], F32, isOutput=True)
    return p


def emit(nc, tc, ctx, params, out_handle=None):
    from contextlib import ExitStack

    x_d = params["x"]
    out_d = params["out"] if out_handle is None else out_handle
    x_ap, out_ap = x_d.ap(), out_d.ap()

    # ---- persistent pools --------------------------------------------
    consts = ctx.enter_context(tc.tile_pool(name="consts", bufs=1))
    wsb_p = ctx.enter_context(tc.tile_pool(name="wsb", bufs=1))
    xn_p = ctx.enter_context(tc.tile_pool(name="xn", bufs=NT))
    xq_p = ctx.enter_context(tc.tile_pool(name="xq", bufs=KC2))
    vT_p = ctx.enter_context(tc.tile_pool(name="vT", bufs=KC2))
    a_p = ctx.enter_context(tc.tile_pool(name="a", bufs=KC2))
    qk_p = ctx.enter_context(tc.tile_pool(name="qk", bufs=6))
    ex_p = ctx.enter_context(tc.tile_pool(name="ex", bufs=3))
    # PSUM budget: m1 2x2 banks + ps2 2 banks + spare 2 banks = 8
    m1_p = ctx.enter_context(
        tc.tile_pool(name="m1p", bufs=2, space=bass.MemorySpace.PSUM))
    ps2_p = ctx.enter_context(
        tc.tile_pool(name="ps2p", bufs=2, space=bass.MemorySpace.PSUM))
    sp_p = ctx.enter_context(
        tc.tile_pool(name="spp", bufs=2, space=bass.MemorySpace.PSUM))

    # ---- weight tiles; DMAs are emitted after the x-tile DMAs so x
    # owns the head of the sync/gpsimd queues --------------------------
    qw_sb = wsb_p.tile([128, PAIRS * KC2, 256], FP8, tag="qw", name="qw_sb")
    kw_sb = wsb_p.tile([128, PAIRS * KC2, 256], FP8, tag="kw", name="kw_sb")
    vw_sb = wsb_p.tile([128, 2 * KC2, 1024], FP8, tag="vw", name="vw_sb")
    pw_sb = wsb_p.tile([128, NT * KC2, 256], FP8, tag="pw", name="pw_sb")

    def emit_weight_dmas():
        # q/k behind x on sync/gpsimd (per-queue serial, so x keeps the
        # early HBM bandwidth); v/p + late consts ride the idle ACT queue.
        for eng, dst, srcd in ((nc.sync, qw_sb, params["q_w8"]),
                               (nc.gpsimd, kw_sb, params["k_w8"])):
            eng.dma_start(
                out=dst,
                in_=srcd.ap().rearrange("p (t f) -> p t f", f=dst.shape[2]))

    def qwt(j, kc2):
        return qw_sb[:, j * KC2 + kc2, :].rearrange("p (i f) -> p i f", f=128)

    def kwt(j, kc2):
        return kw_sb[:, j * KC2 + kc2, :].rearrange("p (i f) -> p i f", f=128)

    def vwt(vhalf, kc2):
        return vw_sb[:, vhalf * KC2 + kc2, :].rearrange(
            "p (i f) -> p i f", f=512)

    def pwt(m, kc2):
        return pw_sb[:, m * KC2 + kc2, :].rearrange("p (i f) -> p i f", f=128)

    def load_const(dram, shape, tag, eng=None):
        t = consts.tile(shape, F32, tag=tag, name=tag)
        (eng or nc.sync).dma_start(out=t, in_=dram.ap())
        return t

    ag_sb = load_const(params["A_grp"], [128, 4], "ag")
    a2_sb = load_const(params["A2T"], [4, 128], "a2")
    # v/p weights on the ACT queue: issued at t=0, done long before use
    for dst, srcd in ((vw_sb, params["v_w8"]), (pw_sb, params["p_w8"])):
        nc.scalar.dma_start(
            out=dst,
            in_=srcd.ap().rearrange("p (t f) -> p t f", f=dst.shape[2]))
    qb_sb = load_const(params["q_b"], [128, NT], "qb", nc.scalar)
    kb_sb = load_const(params["k_b"], [128, NT], "kb", nc.scalar)
    pb_sb = load_const(params["proj_beff"], [128, NT], "pb", nc.scalar)
    nw_sb = load_const(params["norm_w_c"], [128, NT], "nw", nc.scalar)
    nb_sb = load_const(params["norm_b_c"], [128, NT], "nb", nc.scalar)
    onesg = consts.tile([128, 2 * H], F32, tag="onesg", name="onesg")
    nc.vector.memset(onesg, 1.0)
    eps_sb = consts.tile([4, 1], F32, tag="eps", name="eps")
    nc.vector.memset(eps_sb, EPS)
    ebias = consts.tile([128, 1], F32, tag="ebias", name="ebias")
    nc.vector.memset(ebias, EXP_BIAS)

    xq = []   # KC2 x [128, 2, L] fp8 DoubleRow-layout normalized x
    xn = []   # NT x [128, L] f32 residual
    for kc2 in range(KC2):
        t = xq_p.tile([128, 2, L], FP8, tag="xq_t", name="xq_t")
        xq.append(t)
    # a tiles in DoubleRow layout for proj: a[kc2][:, i, :] = pair 2*kc2+i
    abuf = []
    for kc2 in range(KC2):
        t = a_p.tile([128, 2, L], FP8, tag="a_t", name="a_t")
        abuf.append(t)

    # ================= Phase 1: GroupNorm =============================
    # Per-tile: DMA, row-sum (DVE), Square+accum (ACT, one table set),
    # tiny group-reduce matmul into a shared [4, 8, 2] stats bank.
    # Then ONE batched Ln/Exp + scale/bias algebra for all tiles.
    with ExitStack() as ph1:
        xp = ph1.enter_context(tc.tile_pool(name="xp", bufs=NT))
        scr_p = ph1.enter_context(tc.tile_pool(name="scr", bufs=2))
        gn_p = ph1.enter_context(tc.tile_pool(name="gn", bufs=1))

        inv_n = 1.0 / (32 * L)
        gstat = sp_p.tile([4, NT, 2], F32, tag="sp", name="gstat")
        xt_all = []
        for t in range(NT):
            xt = xp.tile([128, L], F32, tag="x_t", name="x_t")
            eng = nc.sync if t % 2 == 0 else nc.gpsimd
            eng.dma_start(out=xt, in_=x_ap[t * 128:(t + 1) * 128, :])
            xt_all.append(xt)
        emit_weight_dmas()
        for t in range(NT):
            xt = xt_all[t]

            stats = gn_p.tile([128, NT, 2], F32, tag="stats", name="stats")
            nc.vector.reduce_sum(
                out=stats[:, t, 0:1], in_=xt, axis=mybir.AxisListType.X)
            scr = scr_p.tile([128, L], F32, tag="scr", name="scr")
            nc.scalar.activation(
                out=scr, in_=xt,
                func=mybir.ActivationFunctionType.Square,
                accum_out=stats[:, t, 1:2])
            nc.tensor.matmul(gstat[:, t, :], ag_sb, stats[:, t, :])

        # batched group-norm algebra over all 32 groups at once
        gs_sb = gn_p.tile([4, NT, 2], F32, tag="gs", name="gs_sb")
        nc.vector.tensor_scalar_mul(out=gs_sb, in0=gstat, scalar1=inv_n)
        m2 = gn_p.tile([4, NT], F32, tag="m2", name="m2")
        nc.vector.tensor_tensor(out=m2, in0=gs_sb[:, :, 0],
                                in1=gs_sb[:, :, 0], op=mybir.AluOpType.mult)
        mi2 = gn_p.tile([4, 2, NT], F32, tag="mi2", name="mi2")
        nc.vector.tensor_copy(out=mi2[:, 0, :], in_=gs_sb[:, :, 0])
        var = gn_p.tile([4, NT], F32, tag="var", name="var")
        nc.vector.tensor_tensor(out=var, in0=gs_sb[:, :, 1], in1=m2,
                                op=mybir.AluOpType.subtract)
        lnv = gn_p.tile([4, NT], F32, tag="lnv", name="lnv")
        nc.scalar.activation(out=lnv, in_=var,
                             func=mybir.ActivationFunctionType.Ln,
                             bias=eps_sb, scale=1.0)
        nc.scalar.activation(out=mi2[:, 1, :], in_=lnv,
                             func=mybir.ActivationFunctionType.Exp,
                             scale=-0.5)
        bc = sp_p.tile([128, 2, NT], F32, tag="sp", name="bc")
        nc.tensor.matmul(bc, a2_sb, mi2)

        scale_all = gn_p.tile([128, NT], F32, tag="scale", name="scale_all")
        nc.vector.tensor_tensor(out=scale_all, in0=nw_sb, in1=bc[:, 1, :],
                                op=mybir.AluOpType.mult)
        tmp = gn_p.tile([128, NT], F32, tag="tmp", name="tmp")
        nc.vector.tensor_tensor(out=tmp, in0=bc[:, 0, :], in1=scale_all,
                                op=mybir.AluOpType.mult)
        bias_all = gn_p.tile([128, NT], F32, tag="bias", name="bias_all")
        nc.vector.tensor_tensor(out=bias_all, in0=nb_sb, in1=tmp,
                                op=mybir.AluOpType.subtract)

        for t in range(NT):
            nc.vector.tensor_scalar(
                out=xq[t // 2][:, t % 2, :], in0=xt_all[t],
                scalar1=scale_all[:, t:t + 1], scalar2=bias_all[:, t:t + 1],
                op0=mybir.AluOpType.mult, op1=mybir.AluOpType.add)
            xnt = xn_p.tile([128, L], F32, tag="xn_t", name="xn_t")
            nc.gpsimd.tensor_scalar(
                out=xnt, in0=xt_all[t],
                scalar1=scale_all[:, t:t + 1], scalar2=bias_all[:, t:t + 1],
                op0=mybir.AluOpType.mult, op1=mybir.AluOpType.add)
            xn.append(xnt)

        # ============= Phase 2: v^T (fp8 DR, swapped operands) ========
        vT2 = []
        for m in range(KC2):
            vt = vT_p.tile([128, 2, H * (CH + 1)], FP8, tag="vT_t",
                           name="vT_t")
            nc.vector.tensor_copy(
                out=vt.rearrange("p i (h c) -> p i h c", c=CH + 1)[:, :, :,
                                                                  CH:CH + 1],
                in_=onesg.rearrange("p (i h o) -> p i h o", i=2, o=1))
            vT2.append(vt)
        for m in range(KC2):
            for i_lc in range(2):
                lc = 2 * m + i_lc
                for vhalf in range(2):
                    acc = sp_p.tile([128, 512], F32, tag="sp", name="vacc")
                    for kc2 in range(KC2):
                        nc.tensor.matmul(
                            acc,
                            xq[kc2][:, :, lc * 128:(lc + 1) * 128],
                            vwt(vhalf, kc2),
                            start=(kc2 == 0), stop=(kc2 == KC2 - 1),
                            perf_mode=DR)
                    nc.vector.tensor_copy(
                        out=vT2[m].rearrange(
                            "p i (h c) -> p i h c", c=CH + 1)[
                                :, i_lc, 8 * vhalf:8 * vhalf + 8, 0:CH],
                        in_=acc.rearrange("p (h c) -> p h c", c=CH))

    # ============ Phase 3: attention with next-pair qk interleaved ====
    pp_p = ctx.enter_context(tc.tile_pool(name="ppp", bufs=NT * LT))
    partials = {}

    def pp_gen():
        """proj partial sums over kc2 0..2 (pairs 0..5 / abuf[0..2]),
        with proj bias and the x_norm residual folded in; phase 4 then
        only adds the last kc2 step."""
        for mo in range(NT):
            for n in range(LT):
                acc = sp_p.tile([128, 512], F32, tag="sp", name="ppacc")
                for kc2 in range(KC2 - 1):
                    nc.tensor.matmul(
                        acc, pwt(mo, kc2),
                        abuf[kc2][:, :, n * 512:(n + 1) * 512],
                        start=(kc2 == 0), stop=(kc2 == KC2 - 2),
                        perf_mode=DR)
                pt = pp_p.tile([128, 512], F32, tag="pp", name="pp")
                nc.vector.scalar_tensor_tensor(
                    out=pt, in0=acc, scalar=pb_sb[:, mo:mo + 1],
                    in1=xn[mo][:, n * 512:(n + 1) * 512],
                    op0=mybir.AluOpType.add, op1=mybir.AluOpType.add)
                partials[(mo, n)] = pt
                yield

    qk_res = {}

    def qk_gen(j):
        """Emit pair j's q/k projection (fp8 DR) in chunks."""
        for name, wfun, b_sb in (("q", qwt, qb_sb), ("k", kwt, kb_sb)):
            dst = qk_p.tile([128, L], BF16, tag=f"{name}_j", name=f"{name}_j")
            for n in range(LT):
                acc = sp_p.tile([128, 512], F32, tag="sp", name="qkacc")
                for kc2 in range(KC2):
                    nc.tensor.matmul(
                        acc, wfun(j, kc2),
                        xq[kc2][:, :, n * 512:(n + 1) * 512],
                        start=(kc2 == 0), stop=(kc2 == KC2 - 1),
                        perf_mode=DR)
                    if kc2 % 2 == 1:
                        yield
                nc.vector.tensor_scalar_add(
                    out=dst[:, n * 512:(n + 1) * 512], in0=acc,
                    scalar1=b_sb[:, j:j + 1])
                yield
            qk_res.setdefault(j, []).append(dst)

    for _ in qk_gen(0):
        pass

    with ExitStack() as ph3:
        rc_p = ph3.enter_context(tc.tile_pool(name="rcp", bufs=4))

        ppg = pp_gen()
        for j in range(PAIRS):
            nxt = qk_gen(j + 1) if j + 1 < PAIRS else None
            pdrive = ppg if j >= PAIRS - 2 else None
            q_j, k_j = qk_res.pop(j)

            for tcn in range(LT):
                ps2 = [ps2_p.tile([CH + 1, 512], F32, tag="ps2",
                                  name=f"ps2_{par}") for par in range(2)]
                ex = None
                exs = {}

                def emit_mm2(m):
                    exr = exs[m].rearrange("p (s c) f -> p c s f", c=2)
                    vtr = vT2[m].rearrange("p i (h c) -> p i h c",
                                           c=CH + 1)
                    for par in range(2):
                        h = 2 * j + par
                        nc.tensor.matmul(
                            ps2[par],
                            vtr[:, :, h, :],
                            exr[:, par, :, :],
                            start=(m == 0), stop=(m == KC2 - 1),
                            perf_mode=DR)

                for sc in range(NT):
                    m1 = m1_p.tile([128, 2, 512], F32, tag="m1", name="m1")
                    for par in range(2):
                        base = CH * par
                        nc.tensor.matmul(
                            m1[:, par, :],
                            k_j[base:base + CH, sc * 128:(sc + 1) * 128],
                            q_j[base:base + CH, tcn * 512:(tcn + 1) * 512])
                    sch = sc % 2
                    if sch == 0:
                        ex = ex_p.tile([128, 4, 512], FP8, tag="ex",
                                       name="ex")
                        exs[sc // 2] = ex
                    nc.scalar.activation(
                        out=ex[:, 2 * sch:2 * sch + 2, :], in_=m1,
                        func=mybir.ActivationFunctionType.Exp,
                        bias=ebias, scale=0.125)
                    if nxt is not None:
                        next(nxt, None)
                    if pdrive is not None:
                        next(pdrive, None)
                    # mm2(m) is emitted only after mm1(sc=2m+3): by the
                    # time the in-order PE queue reaches it, exp(m) has
                    # long completed, so the PE never stalls mid-stream
                    # (stalls keep the PE at its mid p-state).
                    if sc >= 3 and sc % 2 == 1:
                        emit_mm2((sc - 3) // 2)
                emit_mm2(KC2 - 1)
                # epilogue per head: S row to SBUF (recip must not read
                # PSUM: misreads on HW), reciprocal, broadcast, then
                # multiply the raw PSUM rows directly into the fp8 a-tile.
                for par in range(2):
                    s_sb = rc_p.tile([1, 512], F32, tag="ssb", name="s_sb")
                    nc.vector.tensor_copy(out=s_sb,
                                          in_=ps2[par][CH:CH + 1, :])
                    rc = rc_p.tile([1, 512], F32, tag="rc", name="rc")
                    nc.vector.reciprocal_approx_fast(out=rc, in_=s_sb)
                    sbb = rc_p.tile([CH, 512], F32, tag="sbb", name="sbb")
                    nc.gpsimd.partition_broadcast(sbb, rc, channels=CH)
                    nc.vector.tensor_tensor(
                        out=abuf[j // 2][CH * par:CH * par + CH, j % 2,
                                         tcn * 512:(tcn + 1) * 512],
                        in0=ps2[par][0:CH, :], in1=sbb,
                        op=mybir.AluOpType.mult)
            if nxt is not None:
                for _ in nxt:
                    pass
        for _ in ppg:
            pass

    # ========== Phase 4: last proj step + partial + DMA out ===========
    with ExitStack() as ph4:
        out_p = ph4.enter_context(tc.tile_pool(name="outp", bufs=4))
        for mo in range(NT):
            for n in range(LT):
                acc = sp_p.tile([128, 512], F32, tag="sp", name="pacc")
                nc.tensor.matmul(
                    acc, pwt(mo, KC2 - 1),
                    abuf[KC2 - 1][:, :, n * 512:(n + 1) * 512],
                    start=True, stop=True, perf_mode=DR)
                o_sb = out_p.tile([128, 512], F32, tag="o_sb", name="o_sb")
                nc.vector.tensor_tensor(
                    out=o_sb, in0=acc, in1=partials[(mo, n)],
                    op=mybir.AluOpType.add)
                deng = nc.sync if (mo * LT + n) % 2 == 0 else nc.gpsimd
                deng.dma_start(
                    out=out_ap[mo * 128:(mo + 1) * 128,
                               n * 512:(n + 1) * 512],
                    in_=o_sb)


_CACHED = {}


def build_program(repeats=1):
    key = ("nc", repeats)
    if key in _CACHED:
        return _CACHED[key]
    from contextlib import ExitStack

    nc = bacc.Bacc("TRN2", target_bir_lowering=False, debug=False)
    with tile.TileContext(nc) as tc:
        params = declare_params(nc)
        for rep in range(repeats):
            out_h = None
            if rep > 0:
                out_h = nc.dram_tensor(f"out_scratch{rep}", [C, L], F32)
            with ExitStack() as ctx:
                emit(nc, tc, ctx, params, out_h)
    nc.compile()
    _CACHED[key] = nc
    return nc


def host_pack(norm_w, norm_b, qkv_w, qkv_b, proj_w, proj_b):
    """Precompute packed weight layouts (plain numpy)."""
    f = np.float32
    qkv_w = np.asarray(qkv_w, f)
    qkv_b = np.asarray(qkv_b, f)
    proj_w = np.asarray(proj_w, f)
    proj_b = np.asarray(proj_b, f)

    # pair-packed output index maps (tile j: head 2j at 0:64, 2j+1 at 64:128)
    idx_q = np.empty(C, np.int64)
    idx_k = np.empty(C, np.int64)
    for j in range(PAIRS):
        for p in range(128):
            h = 2 * j + p // CH
            i = p % CH
            idx_q[j * 128 + p] = 192 * h + i
            idx_k[j * 128 + p] = 192 * h + CH + i
    idx_v = np.empty(C, np.int64)
    for h in range(H):
        idx_v[CH * h:CH * (h + 1)] = 192 * h + 2 * CH + np.arange(CH)

    # DoubleRow lhsT packs: tile (o, kc2)[p_c, i_c, col] =
    #   w[row_idx[o, col], 256*kc2 + 128*i_c + p_c]
    def pack_dr(w, row_idx, n_out_tiles, out_w):
        nt = n_out_tiles
        res = np.empty((nt, KC2, 128, 2, out_w), f)
        for o in range(nt):
            rows = w[row_idx[o]]  # [out_w, C]
            blk = rows.reshape(out_w, KC2, 2, 128)  # c = 256*kc2+128*i+p
            res[o] = blk.transpose(1, 3, 2, 0)  # [KC2, 128, 2, out_w]
        # partition-major: [128, (o, kc2, 2, out_w)] contiguous per row
        return np.ascontiguousarray(
            res.transpose(2, 0, 1, 3, 4).reshape(128, nt * KC2 * 2 * out_w)
        ).astype(NPFP8)

    q_w8 = pack_dr(qkv_w, idx_q.reshape(PAIRS, 128), PAIRS, 128)
    k_w8 = pack_dr(qkv_w, idx_k.reshape(PAIRS, 128), PAIRS, 128)
    v_w8 = pack_dr(qkv_w, idx_v.reshape(2, 512), 2, 512)
    p_w8 = pack_dr(proj_w, np.arange(C).reshape(NT, 128), NT, 128)

    q_b = np.ascontiguousarray(qkv_b[idx_q].reshape(NT, 128).T)
    k_b = np.ascontiguousarray(qkv_b[idx_k].reshape(NT, 128).T)
    pbe = proj_b + proj_w @ qkv_b[idx_v]
    proj_beff = np.ascontiguousarray(pbe.astype(f).reshape(NT, 128).T)

    norm_w_c = np.ascontiguousarray(np.asarray(norm_w, f).reshape(NT, 128).T)
    norm_b_c = np.ascontiguousarray(np.asarray(norm_b, f).reshape(NT, 128).T)

    pp = np.arange(128)
    A_grp = (pp[:, None] // 32 == np.arange(4)[None, :]).astype(f)
    A2T = np.ascontiguousarray(A_grp.T)

    return dict(
        q_w8=q_w8, k_w8=k_w8, v_w8=v_w8, p_w8=p_w8,
        q_b=q_b, k_b=k_b, proj_beff=proj_beff,
        norm_w_c=norm_w_c, norm_b_c=norm_b_c, A_grp=A_grp, A2T=A2T,
    )


def kernel(x, norm_w, norm_b, qkv_w, qkv_b, proj_w, proj_b, _trace=False):
    x = np.asarray(x, np.float32)
    shared = host_pack(norm_w, norm_b, qkv_w, qkv_b, proj_w, proj_b)
    nc = build_program()
    in_maps = [dict(shared, x=np.ascontiguousarray(x[i])) for i in range(B)]
    res = run_bass_kernel_spmd(nc, in_maps, list(range(B)), trace=_trace)
    out = np.stack([res.results[i]["out"] for i in range(B)], axis=0)
    if _trace:
        kernel._last_results = res
    return out.astype(np.float32)


# revision 17
# speedup vs baseline: 1.0195x; 1.0195x over previous
"""Trainium2 Bass kernel for nn_AttentionBlock (B=8, C=1024, L=1024, H=16, G=32).

Data-parallel over batch: one sample per NeuronCore, no collectives.
Per-core structure (v3 — fp8 DoubleRow for qkv/v/mm2/proj, bf16 mm1):

  1. GroupNorm, pipelined per 128-channel tile (each tile holds exactly 4
     groups, so stats never cross tiles): DVE row-sum + ACT Square-accum
     per tile feeding tiny per-tile selector matmuls into one [4, 8, 2]
     stats bank; the Ln/Exp rsqrt and the scale/bias algebra run ONCE,
     batched over all tiles (avoids per-tile ACT table swaps between the
     Square set and the Ln/Exp set).  Apply writes fp8 DoubleRow-layout
     tiles xq[kc2] = [128, 2, L] (channel c = 256*kc2 + 128*i + p) plus
     f32 residual tiles.
  2. q/k projections: fp8 DoubleRow matmuls (contraction 256/step),
     bias added on the PSUM->SBUF copy, output bf16 pair-packed [128, L]
     (head 2j on partitions 0:64, 2j+1 on 64:128).  v^T is produced
     directly in [L, (h, 65)] fp8 layout by swapping operands (lhsT =
     xq l-slice); a constant ones column per head feeds the softmax
     denominator through mm2.
  3. Attention per (pair, tcn): loop sc: two bf16 mm1s (the heads run
     concurrently on PE row groups 0/64) into a double-buffered
     [128, 2, 512] PSUM tile; one ACT exp(z/8 - 2ln2) -> fp8 slice of a
     [128, (sch, par), 512] tile (e4m3 max 240, max ex ~101; the shift
     cancels in normalization); per sc-pair two fp8 DoubleRow mm2s
     (contraction 256 = two s-chunks) accumulate [a_raw; S].  Epilogue:
     copy S row + a_raw to SBUF (fast bank release; also
     reciprocal_approx_fast misreads PSUM on HW), reciprocal, gpsimd
     partition-broadcast, one DVE multiply into the fp8 a-tile.  The
     next pair's q/k projection interleaves through a generator.
  4. proj: fp8 DoubleRow + (bias_eff + x_norm) residual epilogue, DMA out.

Weights are repacked host-side into DoubleRow lhsT layouts; v-bias is
folded into the proj bias (softmax rows sum to 1).
"""

import numpy as np
import ml_dtypes

import concourse.bass as bass
import concourse.bacc as bacc
import concourse.tile as tile
from concourse import mybir
from concourse.bass_utils import run_bass_kernel_spmd

F32 = mybir.dt.float32
BF16 = mybir.dt.bfloat16
FP8 = mybir.dt.float8e4
DR = mybir.MatmulPerfMode.DoubleRow
NPFP8 = ml_dtypes.float8_e4m3  # matches mybir.dt.float8e4 (IEEE, max 240)

B, C, L, H = 8, 1024, 1024, 16
GROUPS = 32
CH = C // H          # 64 per-head channels
EPS = 1e-5
NT = C // 128        # 8 channel tiles
KC2 = 4              # DoubleRow contraction steps (256 channels each)
LT = L // 512        # 2 free-dim chunks of 512
PAIRS = H // 2       # 8 head pairs
EXP_BIAS = -1.3862944  # -2*ln2: max exp(z/8-2ln2) ~ 101 < 240 (e4m3 max)


def declare_params(nc):
    p = {}
    p["x"] = nc.declare_dram_parameter("x", [C, L], F32, isOutput=False)
    # DoubleRow lhsT packs, partition-major: [128, (otile, kc2, 2, out)]
    p["q_w8"] = nc.declare_dram_parameter("q_w8", [128, PAIRS * KC2 * 256],
                                          FP8, isOutput=False)
    p["k_w8"] = nc.declare_dram_parameter("k_w8", [128, PAIRS * KC2 * 256],
                                          FP8, isOutput=False)
    p["v_w8"] = nc.declare_dram_parameter("v_w8", [128, 2 * KC2 * 1024],
                                          FP8, isOutput=False)
    p["p_w8"] = nc.declare_dram_parameter("p_w8", [128, NT * KC2 * 256],
                                          FP8, isOutput=False)
    p["q_b"] = nc.declare_dram_parameter("q_b", [128, NT], F32, isOutput=False)
    p["k_b"] = nc.declare_dram_parameter("k_b", [128, NT], F32, isOutput=False)
    p["proj_beff"] = nc.declare_dram_parameter("proj_beff", [128, NT], F32,
                                               isOutput=False)
    p["norm_w_c"] = nc.declare_dram_parameter("norm_w_c", [128, NT], F32,
                                              isOutput=False)
    p["norm_b_c"] = nc.declare_dram_parameter("norm_b_c", [128, NT], F32,
                                              isOutput=False)
    p["A_grp"] = nc.declare_dram_parameter("A_grp", [128, 4], F32,
                                           isOutput=False)
    p["A2T"] = nc.declare_dram_parameter("A2T", [4, 128], F32, isOutput=False)
    p["out"] = nc.declare_dram_parameter("out", [C, L], F32, isOutput=True)
    return p


def emit(nc, tc, ctx, params, out_handle=None):
    from contextlib import ExitStack

    x_d = params["x"]
    out_d = params["out"] if out_handle is None else out_handle
    x_ap, out_ap = x_d.ap(), out_d.ap()

    # ---- persistent pools --------------------------------------------
    consts = ctx.enter_context(tc.tile_pool(name="consts", bufs=1))
    wsb_p = ctx.enter_context(tc.tile_pool(name="wsb", bufs=1))
    xn_p = ctx.enter_context(tc.tile_pool(name="xn", bufs=NT))
    xq_p = ctx.enter_context(tc.tile_pool(name="xq", bufs=KC2))
    vT_p = ctx.enter_context(tc.tile_pool(name="vT", bufs=KC2))
    a_p = ctx.enter_context(tc.tile_pool(name="a", bufs=KC2))
    qk_p = ctx.enter_context(tc.tile_pool(name="qk", bufs=6))
    ex_p = ctx.enter_context(tc.tile_pool(name="ex", bufs=3))
    # PSUM budget: m1 2x2 banks + ps2 2 banks + spare 2 banks = 8
    m1_p = ctx.enter_context(
        tc.tile_pool(name="m1p", bufs=2, space=bass.MemorySpace.PSUM))
    ps2_p = ctx.enter_context(
        tc.tile_pool(name="ps2p", bufs=2, space=bass.MemorySpace.PSUM))
    sp_p = ctx.enter_context(
        tc.tile_pool(name="spp", bufs=2, space=bass.MemorySpace.PSUM))

    # ---- weight tiles; DMAs are emitted after the x-tile DMAs so x
    # owns the head of the sync/gpsimd queues --------------------------
    qw_sb = wsb_p.tile([128, PAIRS * KC2, 256], FP8, tag="qw", name="qw_sb")
    kw_sb = wsb_p.tile([128, PAIRS * KC2, 256], FP8, tag="kw", name="kw_sb")
    vw_sb = wsb_p.tile([128, 2 * KC2, 1024], FP8, tag="vw", name="vw_sb")
    pw_sb = wsb_p.tile([128, NT * KC2, 256], FP8, tag="pw", name="pw_sb")

    def emit_weight_dmas():
        # DMA queues are per-queue serial AND a queued DMA blocks any
        # compute emitted later on that engine's queue -- so the ACT
        # queue carries NO DMAs at all, and weights queue strictly
        # behind x: sync gets q, gpsimd gets v (needed first), then k,
        # then p (needed last).
        for eng, dst, srcd in ((nc.gpsimd, vw_sb, params["v_w8"]),
                               (nc.sync, qw_sb, params["q_w8"]),
                               (nc.gpsimd, kw_sb, params["k_w8"]),
                               (nc.gpsimd, pw_sb, params["p_w8"])):
            eng.dma_start(
                out=dst,
                in_=srcd.ap().rearrange("p (t f) -> p t f", f=dst.shape[2]))

    def qwt(j, kc2):
        return qw_sb[:, j * KC2 + kc2, :].rearrange("p (i f) -> p i f", f=128)

    def kwt(j, kc2):
        return kw_sb[:, j * KC2 + kc2, :].rearrange("p (i f) -> p i f", f=128)

    def vwt(vhalf, kc2):
        return vw_sb[:, vhalf * KC2 + kc2, :].rearrange(
            "p (i f) -> p i f", f=512)

    def pwt(m, kc2):
        return pw_sb[:, m * KC2 + kc2, :].rearrange("p (i f) -> p i f", f=128)

    def load_const(dram, shape, tag, eng=None):
        t = consts.tile(shape, F32, tag=tag, name=tag)
        (eng or nc.sync).dma_start(out=t, in_=dram.ap())
        return t

    ag_sb = load_const(params["A_grp"], [128, 4], "ag")
    a2_sb = load_const(params["A2T"], [4, 128], "a2")
    qb_sb = load_const(params["q_b"], [128, NT], "qb")
    kb_sb = load_const(params["k_b"], [128, NT], "kb")
    pb_sb = load_const(params["proj_beff"], [128, NT], "pb")
    nw_sb = load_const(params["norm_w_c"], [128, NT], "nw")
    nb_sb = load_const(params["norm_b_c"], [128, NT], "nb")
    onesg = consts.tile([128, 2 * H], F32, tag="onesg", name="onesg")
    nc.vector.memset(onesg, 1.0)
    eps_sb = consts.tile([4, 1], F32, tag="eps", name="eps")
    nc.vector.memset(eps_sb, EPS)
    ebias = consts.tile([128, 1], F32, tag="ebias", name="ebias")
    nc.vector.memset(ebias, EXP_BIAS)

    xq = []   # KC2 x [128, 2, L] fp8 DoubleRow-layout normalized x
    xn = []   # NT x [128, L] f32 residual
    for kc2 in range(KC2):
        t = xq_p.tile([128, 2, L], FP8, tag="xq_t", name="xq_t")
        xq.append(t)
    # a tiles in DoubleRow layout for proj: a[kc2][:, i, :] = pair 2*kc2+i
    abuf = []
    for kc2 in range(KC2):
        t = a_p.tile([128, 2, L], FP8, tag="a_t", name="a_t")
        abuf.append(t)

    # ================= Phase 1: GroupNorm =============================
    # Per-tile: DMA, row-sum (DVE), Square+accum (ACT, one table set),
    # tiny group-reduce matmul into a shared [4, 8, 2] stats bank.
    # Then ONE batched Ln/Exp + scale/bias algebra for all tiles.
    with ExitStack() as ph1:
        xp = ph1.enter_context(tc.tile_pool(name="xp", bufs=NT))
        scr_p = ph1.enter_context(tc.tile_pool(name="scr", bufs=2))
        gn_p = ph1.enter_context(tc.tile_pool(name="gn", bufs=1))

        inv_n = 1.0 / (32 * L)
        gstat = sp_p.tile([4, NT, 2], F32, tag="sp", name="gstat")
        xt_all = []
        for t in range(NT):
            xt = xp.tile([128, L], F32, tag="x_t", name="x_t")
            eng = nc.sync if t % 2 == 0 else nc.gpsimd
            eng.dma_start(out=xt, in_=x_ap[t * 128:(t + 1) * 128, :])
            xt_all.append(xt)
        emit_weight_dmas()
        for t in range(NT):
            xt = xt_all[t]

            stats = gn_p.tile([128, NT, 2], F32, tag="stats", name="stats")
            nc.vector.reduce_sum(
                out=stats[:, t, 0:1], in_=xt, axis=mybir.AxisListType.X)
            scr = scr_p.tile([128, L], F32, tag="scr", name="scr")
            nc.scalar.activation(
                out=scr, in_=xt,
                func=mybir.ActivationFunctionType.Square,
                accum_out=stats[:, t, 1:2])
            nc.tensor.matmul(gstat[:, t, :], ag_sb, stats[:, t, :])

        # batched group-norm algebra over all 32 groups at once
        gs_sb = gn_p.tile([4, NT, 2], F32, tag="gs", name="gs_sb")
        nc.vector.tensor_scalar_mul(out=gs_sb, in0=gstat, scalar1=inv_n)
        m2 = gn_p.tile([4, NT], F32, tag="m2", name="m2")
        nc.vector.tensor_tensor(out=m2, in0=gs_sb[:, :, 0],
                                in1=gs_sb[:, :, 0], op=mybir.AluOpType.mult)
        mi2 = gn_p.tile([4, 2, NT], F32, tag="mi2", name="mi2")
        nc.vector.tensor_copy(out=mi2[:, 0, :], in_=gs_sb[:, :, 0])
        var = gn_p.tile([4, NT], F32, tag="var", name="var")
        nc.vector.tensor_tensor(out=var, in0=gs_sb[:, :, 1], in1=m2,
                                op=mybir.AluOpType.subtract)
        lnv = gn_p.tile([4, NT], F32, tag="lnv", name="lnv")
        nc.scalar.activation(out=lnv, in_=var,
                             func=mybir.ActivationFunctionType.Ln,
                             bias=eps_sb, scale=1.0)
        nc.scalar.activation(out=mi2[:, 1, :], in_=lnv,
                             func=mybir.ActivationFunctionType.Exp,
                             scale=-0.5)
        bc = sp_p.tile([128, 2, NT], F32, tag="sp", name="bc")
        nc.tensor.matmul(bc, a2_sb, mi2)

        scale_all = gn_p.tile([128, NT], F32, tag="scale", name="scale_all")
        nc.vector.tensor_tensor(out=scale_all, in0=nw_sb, in1=bc[:, 1, :],
                                op=mybir.AluOpType.mult)
        tmp = gn_p.tile([128, NT], F32, tag="tmp", name="tmp")
        nc.vector.tensor_tensor(out=tmp, in0=bc[:, 0, :], in1=scale_all,
                                op=mybir.AluOpType.mult)
        bias_all = gn_p.tile([128, NT], F32, tag="bias", name="bias_all")
        nc.vector.tensor_tensor(out=bias_all, in0=nb_sb, in1=tmp,
                                op=mybir.AluOpType.subtract)

        for t in range(NT):
            nc.vector.tensor_scalar(
                out=xq[t // 2][:, t % 2, :], in0=xt_all[t],
                scalar1=scale_all[:, t:t + 1], scalar2=bias_all[:, t:t + 1],
                op0=mybir.AluOpType.mult, op1=mybir.AluOpType.add)
            xnt = xn_p.tile([128, L], F32, tag="xn_t", name="xn_t")
            nc.gpsimd.tensor_scalar(
                out=xnt, in0=xt_all[t],
                scalar1=scale_all[:, t:t + 1], scalar2=bias_all[:, t:t + 1],
                op0=mybir.AluOpType.mult, op1=mybir.AluOpType.add)
            xn.append(xnt)

        # ============= Phase 2: v^T (fp8 DR, swapped operands) ========
        vT2 = []
        for m in range(KC2):
            vt = vT_p.tile([128, 2, H * (CH + 1)], FP8, tag="vT_t",
                           name="vT_t")
            nc.vector.tensor_copy(
                out=vt.rearrange("p i (h c) -> p i h c", c=CH + 1)[:, :, :,
                                                                  CH:CH + 1],
                in_=onesg.rearrange("p (i h o) -> p i h o", i=2, o=1))
            vT2.append(vt)
        for m in range(KC2):
            for i_lc in range(2):
                lc = 2 * m + i_lc
                for vhalf in range(2):
                    acc = sp_p.tile([128, 512], F32, tag="sp", name="vacc")
                    for kc2 in range(KC2):
                        nc.tensor.matmul(
                            acc,
                            xq[kc2][:, :, lc * 128:(lc + 1) * 128],
                            vwt(vhalf, kc2),
                            start=(kc2 == 0), stop=(kc2 == KC2 - 1),
                            perf_mode=DR)
                    nc.vector.tensor_copy(
                        out=vT2[m].rearrange(
                            "p i (h c) -> p i h c", c=CH + 1)[
                                :, i_lc, 8 * vhalf:8 * vhalf + 8, 0:CH],
                        in_=acc.rearrange("p (h c) -> p h c", c=CH))

    # ============ Phase 3: attention with next-pair qk interleaved ====
    pp_p = ctx.enter_context(tc.tile_pool(name="ppp", bufs=NT * LT))
    partials = {}

    def pp_gen():
        """proj partial sums over kc2 0..2 (pairs 0..5 / abuf[0..2]),
        with proj bias and the x_norm residual folded in; phase 4 then
        only adds the last kc2 step."""
        for mo in range(NT):
            for n in range(LT):
                acc = sp_p.tile([128, 512], F32, tag="sp", name="ppacc")
                for kc2 in range(KC2 - 1):
                    nc.tensor.matmul(
                        acc, pwt(mo, kc2),
                        abuf[kc2][:, :, n * 512:(n + 1) * 512],
                        start=(kc2 == 0), stop=(kc2 == KC2 - 2),
                        perf_mode=DR)
                pt = pp_p.tile([128, 512], F32, tag="pp", name="pp")
                nc.vector.scalar_tensor_tensor(
                    out=pt, in0=acc, scalar=pb_sb[:, mo:mo + 1],
                    in1=xn[mo][:, n * 512:(n + 1) * 512],
                    op0=mybir.AluOpType.add, op1=mybir.AluOpType.add)
                partials[(mo, n)] = pt
                yield

    qk_res = {}

    def qk_gen(j):
        """Emit pair j's q/k projection (fp8 DR) in chunks."""
        for name, wfun, b_sb in (("q", qwt, qb_sb), ("k", kwt, kb_sb)):
            dst = qk_p.tile([128, L], BF16, tag=f"{name}_j", name=f"{name}_j")
            for n in range(LT):
                acc = sp_p.tile([128, 512], F32, tag="sp", name="qkacc")
                for kc2 in range(KC2):
                    nc.tensor.matmul(
                        acc, wfun(j, kc2),
                        xq[kc2][:, :, n * 512:(n + 1) * 512],
                        start=(kc2 == 0), stop=(kc2 == KC2 - 1),
                        perf_mode=DR)
                    if kc2 % 2 == 1:
                        yield
                nc.vector.tensor_scalar_add(
                    out=dst[:, n * 512:(n + 1) * 512], in0=acc,
                    scalar1=b_sb[:, j:j + 1])
                yield
            qk_res.setdefault(j, []).append(dst)

    for _ in qk_gen(0):
        pass

    with ExitStack() as ph3:
        rc_p = ph3.enter_context(tc.tile_pool(name="rcp", bufs=4))

        ppg = pp_gen()
        for j in range(PAIRS):
            nxt = qk_gen(j + 1) if j + 1 < PAIRS else None
            pdrive = ppg if j >= PAIRS - 2 else None
            q_j, k_j = qk_res.pop(j)

            for tcn in range(LT):
                ps2 = [ps2_p.tile([CH + 1, 512], F32, tag="ps2",
                                  name=f"ps2_{par}") for par in range(2)]
                ex = None
                exs = {}

                def emit_mm2(m):
                    exr = exs[m].rearrange("p (s c) f -> p c s f", c=2)
                    vtr = vT2[m].rearrange("p i (h c) -> p i h c",
                                           c=CH + 1)
                    for par in range(2):
                        h = 2 * j + par
                        nc.tensor.matmul(
                            ps2[par],
                            vtr[:, :, h, :],
                            exr[:, par, :, :],
                            start=(m == 0), stop=(m == KC2 - 1),
                            perf_mode=DR)

                for sc in range(NT):
                    m1 = m1_p.tile([128, 2, 512], F32, tag="m1", name="m1")
                    for par in range(2):
                        base = CH * par
                        nc.tensor.matmul(
                            m1[:, par, :],
                            k_j[base:base + CH, sc * 128:(sc + 1) * 128],
                            q_j[base:base + CH, tcn * 512:(tcn + 1) * 512])
                    sch = sc % 2
                    if sch == 0:
                        ex = ex_p.tile([128, 4, 512], FP8, tag="ex",
                                       name="ex")
                        exs[sc // 2] = ex
                    nc.scalar.activation(
                        out=ex[:, 2 * sch:2 * sch + 2, :], in_=m1,
                        func=mybir.ActivationFunctionType.Exp,
                        bias=ebias, scale=0.125)
                    if nxt is not None:
                        next(nxt, None)
                    if pdrive is not None and sch == 1:
                        next(pdrive, None)
                    # mm2(m) is emitted only after mm1(sc=2m+3): by the
                    # time the in-order PE queue reaches it, exp(m) has
                    # long completed, so the PE never stalls mid-stream
                    # (stalls keep the PE at its mid p-state).
                    if sc >= 3 and sc % 2 == 1:
                        emit_mm2((sc - 3) // 2)
                emit_mm2(KC2 - 1)
                # epilogue per head: S row to SBUF (recip must not read
                # PSUM: misreads on HW), reciprocal, broadcast, then
                # multiply the raw PSUM rows directly into the fp8 a-tile.
                for par in range(2):
                    s_sb = rc_p.tile([1, 512], F32, tag="ssb", name="s_sb")
                    nc.vector.tensor_copy(out=s_sb,
                                          in_=ps2[par][CH:CH + 1, :])
                    rc = rc_p.tile([1, 512], F32, tag="rc", name="rc")
                    nc.vector.reciprocal_approx_fast(out=rc, in_=s_sb)
                    sbb = rc_p.tile([CH, 512], F32, tag="sbb", name="sbb")
                    nc.gpsimd.partition_broadcast(sbb, rc, channels=CH)
                    nc.vector.tensor_tensor(
                        out=abuf[j // 2][CH * par:CH * par + CH, j % 2,
                                         tcn * 512:(tcn + 1) * 512],
                        in0=ps2[par][0:CH, :], in1=sbb,
                        op=mybir.AluOpType.mult)
            if nxt is not None:
                for _ in nxt:
                    pass
        for _ in ppg:
            pass

    # ========== Phase 4: last proj step + partial + DMA out ===========
    with ExitStack() as ph4:
        out_p = ph4.enter_context(tc.tile_pool(name="outp", bufs=4))
        for mo in range(NT):
            for n in range(LT):
                acc = sp_p.tile([128, 512], F32, tag="sp", name="pacc")
                nc.tensor.matmul(
                    acc, pwt(mo, KC2 - 1),
                    abuf[KC2 - 1][:, :, n * 512:(n + 1) * 512],
                    start=True, stop=True, perf_mode=DR)
                o_sb = out_p.tile([128, 512], F32, tag="o_sb", name="o_sb")
                nc.vector.tensor_tensor(
                    out=o_sb, in0=acc, in1=partials[(mo, n)],
                    op=mybir.AluOpType.add)
                deng = nc.sync if (mo * LT + n) % 2 == 0 else nc.gpsimd
                deng.dma_start(
                    out=out_ap[mo * 128:(mo + 1) * 128,
                               n * 512:(n + 1) * 512],
                    in_=o_sb)


_CACHED = {}


def build_program(repeats=1):
    key = ("nc", repeats)
    if key in _CACHED:
        return _CACHED[key]
    from contextlib import ExitStack

    nc = bacc.Bacc("TRN2", target_bir_lowering=False, debug=False)
    with tile.TileContext(nc) as tc:
        params = declare_params(nc)
        for rep in range(repeats):
            out_h = None
            if rep > 0:
                out_h = nc.dram_tensor(f"out_scratch{rep}", [C, L], F32)
            with ExitStack() as ctx:
                emit(nc, tc, ctx, params, out_h)
    nc.compile()
    _CACHED[key] = nc
    return nc


def host_pack(norm_w, norm_b, qkv_w, qkv_b, proj_w, proj_b):
    """Precompute packed weight layouts (plain numpy)."""
    f = np.float32
    qkv_w = np.asarray(qkv_w, f)
    qkv_b = np.asarray(qkv_b, f)
    proj_w = np.asarray(proj_w, f)
    proj_b = np.asarray(proj_b, f)

    # pair-packed output index maps (tile j: head 2j at 0:64, 2j+1 at 64:128)
    idx_q = np.empty(C, np.int64)
    idx_k = np.empty(C, np.int64)
    for j in range(PAIRS):
        for p in range(128):
            h = 2 * j + p // CH
            i = p % CH
            idx_q[j * 128 + p] = 192 * h + i
            idx_k[j * 128 + p] = 192 * h + CH + i
    idx_v = np.empty(C, np.int64)
    for h in range(H):
        idx_v[CH * h:CH * (h + 1)] = 192 * h + 2 * CH + np.arange(CH)

    # DoubleRow lhsT packs: tile (o, kc2)[p_c, i_c, col] =
    #   w[row_idx[o, col], 256*kc2 + 128*i_c + p_c]
    def pack_dr(w, row_idx, n_out_tiles, out_w):
        nt = n_out_tiles
        res = np.empty((nt, KC2, 128, 2, out_w), f)
        for o in range(nt):
            rows = w[row_idx[o]]  # [out_w, C]
            blk = rows.reshape(out_w, KC2, 2, 128)  # c = 256*kc2+128*i+p
            res[o] = blk.transpose(1, 3, 2, 0)  # [KC2, 128, 2, out_w]
        # partition-major: [128, (o, kc2, 2, out_w)] contiguous per row
        return np.ascontiguousarray(
            res.transpose(2, 0, 1, 3, 4).reshape(128, nt * KC2 * 2 * out_w)
        ).astype(NPFP8)

    q_w8 = pack_dr(qkv_w, idx_q.reshape(PAIRS, 128), PAIRS, 128)
    k_w8 = pack_dr(qkv_w, idx_k.reshape(PAIRS, 128), PAIRS, 128)
    v_w8 = pack_dr(qkv_w, idx_v.reshape(2, 512), 2, 512)
    p_w8 = pack_dr(proj_w, np.arange(C).reshape(NT, 128), NT, 128)

    q_b = np.ascontiguousarray(qkv_b[idx_q].reshape(NT, 128).T)
    k_b = np.ascontiguousarray(qkv_b[idx_k].reshape(NT, 128).T)
    pbe = proj_b + proj_w @ qkv_b[idx_v]
    proj_beff = np.ascontiguousarray(pbe.astype(f).reshape(NT, 128).T)

    norm_w_c = np.ascontiguousarray(np.asarray(norm_w, f).reshape(NT, 128).T)
    norm_b_c = np.ascontiguousarray(np.asarray(norm_b, f).reshape(NT, 128).T)

    pp = np.arange(128)
    A_grp = (pp[:, None] // 32 == np.arange(4)[None, :]).astype(f)
    A2T = np.ascontiguousarray(A_grp.T)

    return dict(
        q_w8=q_w8, k_w8=k_w8, v_w8=v_w8, p_w8=p_w8,
        q_b=q_b, k_b=k_b, proj_beff=proj_beff,
        norm_w_c=norm_w_c, norm_b_c=norm_b_c, A_grp=A_grp, A2T=A2T,
    )


def kernel(x, norm_w, norm_b, qkv_w, qkv_b, proj_w, proj_b, _trace=False):
    x = np.asarray(x, np.float32)
    shared = host_pack(norm_w, norm_b, qkv_w, qkv_b, proj_w, proj_b)
    nc = build_program()
    in_maps = [dict(shared, x=np.ascontiguousarray(x[i])) for i in range(B)]
    res = run_bass_kernel_spmd(nc, in_maps, list(range(B)), trace=_trace)
    out = np.stack([res.results[i]["out"] for i in range(B)], axis=0)
    if _trace:
        kernel._last_results = res
    return out.astype(np.float32)


# revision 18
# speedup vs baseline: 1.0241x; 1.0045x over previous
"""Trainium2 Bass kernel for nn_AttentionBlock (B=8, C=1024, L=1024, H=16, G=32).

Data-parallel over batch: one sample per NeuronCore, no collectives.
Per-core structure (v3 — fp8 DoubleRow for qkv/v/mm2/proj, bf16 mm1):

  1. GroupNorm, pipelined per 128-channel tile (each tile holds exactly 4
     groups, so stats never cross tiles): DVE row-sum + ACT Square-accum
     per tile feeding tiny per-tile selector matmuls into one [4, 8, 2]
     stats bank; the Ln/Exp rsqrt and the scale/bias algebra run ONCE,
     batched over all tiles (avoids per-tile ACT table swaps between the
     Square set and the Ln/Exp set).  Apply writes fp8 DoubleRow-layout
     tiles xq[kc2] = [128, 2, L] (channel c = 256*kc2 + 128*i + p) plus
     f32 residual tiles.
  2. q/k projections: fp8 DoubleRow matmuls (contraction 256/step),
     bias added on the PSUM->SBUF copy, output bf16 pair-packed [128, L]
     (head 2j on partitions 0:64, 2j+1 on 64:128).  v^T is produced
     directly in [L, (h, 65)] fp8 layout by swapping operands (lhsT =
     xq l-slice); a constant ones column per head feeds the softmax
     denominator through mm2.
  3. Attention per (pair, tcn): loop sc: two bf16 mm1s (the heads run
     concurrently on PE row groups 0/64) into a double-buffered
     [128, 2, 512] PSUM tile; one ACT exp(z/8 - 2ln2) -> fp8 slice of a
     [128, (sch, par), 512] tile (e4m3 max 240, max ex ~101; the shift
     cancels in normalization); per sc-pair two fp8 DoubleRow mm2s
     (contraction 256 = two s-chunks) accumulate [a_raw; S].  Epilogue:
     copy S row + a_raw to SBUF (fast bank release; also
     reciprocal_approx_fast misreads PSUM on HW), reciprocal, gpsimd
     partition-broadcast, one DVE multiply into the fp8 a-tile.  The
     next pair's q/k projection interleaves through a generator.
  4. proj: fp8 DoubleRow + (bias_eff + x_norm) residual epilogue, DMA out.

Weights are repacked host-side into DoubleRow lhsT layouts; v-bias is
folded into the proj bias (softmax rows sum to 1).
"""

import numpy as np
import ml_dtypes

import concourse.bass as bass
import concourse.bacc as bacc
import concourse.tile as tile
from concourse import mybir
from concourse.bass_utils import run_bass_kernel_spmd

F32 = mybir.dt.float32
BF16 = mybir.dt.bfloat16
FP8 = mybir.dt.float8e4
DR = mybir.MatmulPerfMode.DoubleRow
NPFP8 = ml_dtypes.float8_e4m3  # matches mybir.dt.float8e4 (IEEE, max 240)

B, C, L, H = 8, 1024, 1024, 16
GROUPS = 32
CH = C // H          # 64 per-head channels
EPS = 1e-5
NT = C // 128        # 8 channel tiles
KC2 = 4              # DoubleRow contraction steps (256 channels each)
LT = L // 512        # 2 free-dim chunks of 512
PAIRS = H // 2       # 8 head pairs
EXP_BIAS = -1.3862944  # -2*ln2: max exp(z/8-2ln2) ~ 101 < 240 (e4m3 max)


def declare_params(nc):
    p = {}
    p["x"] = nc.declare_dram_parameter("x", [C, L], F32, isOutput=False)
    # DoubleRow lhsT packs, partition-major: [128, (otile, kc2, 2, out)]
    p["q_w8"] = nc.declare_dram_parameter("q_w8", [128, PAIRS * KC2 * 256],
                                          FP8, isOutput=False)
    p["k_w8"] = nc.declare_dram_parameter("k_w8", [128, PAIRS * KC2 * 256],
                                          FP8, isOutput=False)
    p["v_w8"] = nc.declare_dram_parameter("v_w8", [128, 2 * KC2 * 1024],
                                          FP8, isOutput=False)
    p["p_w8"] = nc.declare_dram_parameter("p_w8", [128, NT * KC2 * 256],
                                          FP8, isOutput=False)
    p["q_b"] = nc.declare_dram_parameter("q_b", [128, NT], F32, isOutput=False)
    p["k_b"] = nc.declare_dram_parameter("k_b", [128, NT], F32, isOutput=False)
    p["proj_beff"] = nc.declare_dram_parameter("proj_beff", [128, NT], F32,
                                               isOutput=False)
    p["norm_w_c"] = nc.declare_dram_parameter("norm_w_c", [128, NT], F32,
                                              isOutput=False)
    p["norm_b_c"] = nc.declare_dram_parameter("norm_b_c", [128, NT], F32,
                                              isOutput=False)
    p["A_grp"] = nc.declare_dram_parameter("A_grp", [128, 4], F32,
                                           isOutput=False)
    p["A2T"] = nc.declare_dram_parameter("A2T", [4, 128], F32, isOutput=False)
    p["out"] = nc.declare_dram_parameter("out", [C, L], F32, isOutput=True)
    return p


def emit(nc, tc, ctx, params, out_handle=None):
    from contextlib import ExitStack

    x_d = params["x"]
    out_d = params["out"] if out_handle is None else out_handle
    x_ap, out_ap = x_d.ap(), out_d.ap()

    # ---- persistent pools --------------------------------------------
    consts = ctx.enter_context(tc.tile_pool(name="consts", bufs=1))
    wsb_p = ctx.enter_context(tc.tile_pool(name="wsb", bufs=1))
    xn_p = ctx.enter_context(tc.tile_pool(name="xn", bufs=NT))
    xq_p = ctx.enter_context(tc.tile_pool(name="xq", bufs=KC2))
    vT_p = ctx.enter_context(tc.tile_pool(name="vT", bufs=KC2))
    a_p = ctx.enter_context(tc.tile_pool(name="a", bufs=KC2))
    qk_p = ctx.enter_context(tc.tile_pool(name="qk", bufs=6))
    ex_p = ctx.enter_context(tc.tile_pool(name="ex", bufs=3))
    # PSUM budget: m1 2x2 banks + ps2 2 banks + spare 2 banks = 8
    m1_p = ctx.enter_context(
        tc.tile_pool(name="m1p", bufs=2, space=bass.MemorySpace.PSUM))
    ps2_p = ctx.enter_context(
        tc.tile_pool(name="ps2p", bufs=2, space=bass.MemorySpace.PSUM))
    sp_p = ctx.enter_context(
        tc.tile_pool(name="spp", bufs=2, space=bass.MemorySpace.PSUM))

    # ---- weight tiles; DMAs are emitted after the x-tile DMAs so x
    # owns the head of the sync/gpsimd queues --------------------------
    qw_sb = wsb_p.tile([128, PAIRS * KC2, 256], FP8, tag="qw", name="qw_sb")
    kw_sb = wsb_p.tile([128, PAIRS * KC2, 256], FP8, tag="kw", name="kw_sb")
    vw_sb = wsb_p.tile([128, 2 * KC2, 1024], FP8, tag="vw", name="vw_sb")
    pw_sb = wsb_p.tile([128, NT * KC2, 256], FP8, tag="pw", name="pw_sb")

    def emit_weight_dmas():
        # The DMA engine pool drains ALL queued transfers at aggregate
        # bandwidth, so co-queued weights would delay x's completion by
        # ~20us.  Gate each weight DMA behind the x transfers: a tiny
        # gpsimd memset on the destination tile is queued after the
        # x-odd DMAs (compute on a queue waits for prior DMA transfer
        # completion), and the weight DMA WAW-depends on that memset.
        for dst in (vw_sb, qw_sb, kw_sb, pw_sb):
            nc.gpsimd.memset(dst[0:1, 0:1, 0:2], 0)
        for eng, dst, srcd in ((nc.gpsimd, vw_sb, params["v_w8"]),
                               (nc.sync, qw_sb, params["q_w8"]),
                               (nc.gpsimd, kw_sb, params["k_w8"]),
                               (nc.gpsimd, pw_sb, params["p_w8"])):
            eng.dma_start(
                out=dst,
                in_=srcd.ap().rearrange("p (t f) -> p t f", f=dst.shape[2]))

    def qwt(j, kc2):
        return qw_sb[:, j * KC2 + kc2, :].rearrange("p (i f) -> p i f", f=128)

    def kwt(j, kc2):
        return kw_sb[:, j * KC2 + kc2, :].rearrange("p (i f) -> p i f", f=128)

    def vwt(vhalf, kc2):
        return vw_sb[:, vhalf * KC2 + kc2, :].rearrange(
            "p (i f) -> p i f", f=512)

    def pwt(m, kc2):
        return pw_sb[:, m * KC2 + kc2, :].rearrange("p (i f) -> p i f", f=128)

    def load_const(dram, shape, tag, eng=None):
        t = consts.tile(shape, F32, tag=tag, name=tag)
        (eng or nc.sync).dma_start(out=t, in_=dram.ap())
        return t

    ag_sb = load_const(params["A_grp"], [128, 4], "ag")
    a2_sb = load_const(params["A2T"], [4, 128], "a2")
    qb_sb = load_const(params["q_b"], [128, NT], "qb")
    kb_sb = load_const(params["k_b"], [128, NT], "kb")
    pb_sb = load_const(params["proj_beff"], [128, NT], "pb")
    nw_sb = load_const(params["norm_w_c"], [128, NT], "nw")
    nb_sb = load_const(params["norm_b_c"], [128, NT], "nb")
    onesg = consts.tile([128, 2 * H], F32, tag="onesg", name="onesg")
    nc.vector.memset(onesg, 1.0)
    eps_sb = consts.tile([4, 1], F32, tag="eps", name="eps")
    nc.vector.memset(eps_sb, EPS)
    ebias = consts.tile([128, 1], F32, tag="ebias", name="ebias")
    nc.vector.memset(ebias, EXP_BIAS)

    xq = []   # KC2 x [128, 2, L] fp8 DoubleRow-layout normalized x
    xn = []   # NT x [128, L] f32 residual
    for kc2 in range(KC2):
        t = xq_p.tile([128, 2, L], FP8, tag="xq_t", name="xq_t")
        xq.append(t)
    # a tiles in DoubleRow layout for proj: a[kc2][:, i, :] = pair 2*kc2+i
    abuf = []
    for kc2 in range(KC2):
        t = a_p.tile([128, 2, L], FP8, tag="a_t", name="a_t")
        abuf.append(t)

    # ================= Phase 1: GroupNorm =============================
    # Per-tile: DMA, row-sum (DVE), Square+accum (ACT, one table set),
    # tiny group-reduce matmul into a shared [4, 8, 2] stats bank.
    # Then ONE batched Ln/Exp + scale/bias algebra for all tiles.
    with ExitStack() as ph1:
        xp = ph1.enter_context(tc.tile_pool(name="xp", bufs=NT))
        scr_p = ph1.enter_context(tc.tile_pool(name="scr", bufs=2))
        gn_p = ph1.enter_context(tc.tile_pool(name="gn", bufs=1))

        inv_n = 1.0 / (32 * L)
        gstat = sp_p.tile([4, NT, 2], F32, tag="sp", name="gstat")
        xt_all = []
        for t in range(NT):
            xt = xp.tile([128, L], F32, tag="x_t", name="x_t")
            eng = nc.sync if t % 2 == 0 else nc.gpsimd
            eng.dma_start(out=xt, in_=x_ap[t * 128:(t + 1) * 128, :])
            xt_all.append(xt)
        emit_weight_dmas()
        for t in range(NT):
            xt = xt_all[t]

            stats = gn_p.tile([128, NT, 2], F32, tag="stats", name="stats")
            nc.vector.reduce_sum(
                out=stats[:, t, 0:1], in_=xt, axis=mybir.AxisListType.X)
            scr = scr_p.tile([128, L], F32, tag="scr", name="scr")
            nc.scalar.activation(
                out=scr, in_=xt,
                func=mybir.ActivationFunctionType.Square,
                accum_out=stats[:, t, 1:2])
            nc.tensor.matmul(gstat[:, t, :], ag_sb, stats[:, t, :])

        # batched group-norm algebra over all 32 groups at once
        gs_sb = gn_p.tile([4, NT, 2], F32, tag="gs", name="gs_sb")
        nc.vector.tensor_scalar_mul(out=gs_sb, in0=gstat, scalar1=inv_n)
        m2 = gn_p.tile([4, NT], F32, tag="m2", name="m2")
        nc.vector.tensor_tensor(out=m2, in0=gs_sb[:, :, 0],
                                in1=gs_sb[:, :, 0], op=mybir.AluOpType.mult)
        mi2 = gn_p.tile([4, 2, NT], F32, tag="mi2", name="mi2")
        nc.vector.tensor_copy(out=mi2[:, 0, :], in_=gs_sb[:, :, 0])
        var = gn_p.tile([4, NT], F32, tag="var", name="var")
        nc.vector.tensor_tensor(out=var, in0=gs_sb[:, :, 1], in1=m2,
                                op=mybir.AluOpType.subtract)
        lnv = gn_p.tile([4, NT], F32, tag="lnv", name="lnv")
        nc.scalar.activation(out=lnv, in_=var,
                             func=mybir.ActivationFunctionType.Ln,
                             bias=eps_sb, scale=1.0)
        nc.scalar.activation(out=mi2[:, 1, :], in_=lnv,
                             func=mybir.ActivationFunctionType.Exp,
                             scale=-0.5)
        bc = sp_p.tile([128, 2, NT], F32, tag="sp", name="bc")
        nc.tensor.matmul(bc, a2_sb, mi2)

        scale_all = gn_p.tile([128, NT], F32, tag="scale", name="scale_all")
        nc.vector.tensor_tensor(out=scale_all, in0=nw_sb, in1=bc[:, 1, :],
                                op=mybir.AluOpType.mult)
        tmp = gn_p.tile([128, NT], F32, tag="tmp", name="tmp")
        nc.vector.tensor_tensor(out=tmp, in0=bc[:, 0, :], in1=scale_all,
                                op=mybir.AluOpType.mult)
        bias_all = gn_p.tile([128, NT], F32, tag="bias", name="bias_all")
        nc.vector.tensor_tensor(out=bias_all, in0=nb_sb, in1=tmp,
                                op=mybir.AluOpType.subtract)

        for t in range(NT):
            nc.vector.tensor_scalar(
                out=xq[t // 2][:, t % 2, :], in0=xt_all[t],
                scalar1=scale_all[:, t:t + 1], scalar2=bias_all[:, t:t + 1],
                op0=mybir.AluOpType.mult, op1=mybir.AluOpType.add)
            xnt = xn_p.tile([128, L], F32, tag="xn_t", name="xn_t")
            nc.gpsimd.tensor_scalar(
                out=xnt, in0=xt_all[t],
                scalar1=scale_all[:, t:t + 1], scalar2=bias_all[:, t:t + 1],
                op0=mybir.AluOpType.mult, op1=mybir.AluOpType.add)
            xn.append(xnt)

        # ============= Phase 2: v^T (fp8 DR, swapped operands) ========
        vT2 = []
        for m in range(KC2):
            vt = vT_p.tile([128, 2, H * (CH + 1)], FP8, tag="vT_t",
                           name="vT_t")
            nc.vector.tensor_copy(
                out=vt.rearrange("p i (h c) -> p i h c", c=CH + 1)[:, :, :,
                                                                  CH:CH + 1],
                in_=onesg.rearrange("p (i h o) -> p i h o", i=2, o=1))
            vT2.append(vt)
        for m in range(KC2):
            for i_lc in range(2):
                lc = 2 * m + i_lc
                for vhalf in range(2):
                    acc = sp_p.tile([128, 512], F32, tag="sp", name="vacc")
                    for kc2 in range(KC2):
                        nc.tensor.matmul(
                            acc,
                            xq[kc2][:, :, lc * 128:(lc + 1) * 128],
                            vwt(vhalf, kc2),
                            start=(kc2 == 0), stop=(kc2 == KC2 - 1),
                            perf_mode=DR)
                    nc.vector.tensor_copy(
                        out=vT2[m].rearrange(
                            "p i (h c) -> p i h c", c=CH + 1)[
                                :, i_lc, 8 * vhalf:8 * vhalf + 8, 0:CH],
                        in_=acc.rearrange("p (h c) -> p h c", c=CH))

    # ============ Phase 3: attention with next-pair qk interleaved ====
    pp_p = ctx.enter_context(tc.tile_pool(name="ppp", bufs=NT * LT))
    partials = {}

    def pp_gen():
        """proj partial sums over kc2 0..2 (pairs 0..5 / abuf[0..2]),
        with proj bias and the x_norm residual folded in; phase 4 then
        only adds the last kc2 step."""
        for mo in range(NT):
            for n in range(LT):
                acc = sp_p.tile([128, 512], F32, tag="sp", name="ppacc")
                for kc2 in range(KC2 - 1):
                    nc.tensor.matmul(
                        acc, pwt(mo, kc2),
                        abuf[kc2][:, :, n * 512:(n + 1) * 512],
                        start=(kc2 == 0), stop=(kc2 == KC2 - 2),
                        perf_mode=DR)
                pt = pp_p.tile([128, 512], F32, tag="pp", name="pp")
                nc.vector.scalar_tensor_tensor(
                    out=pt, in0=acc, scalar=pb_sb[:, mo:mo + 1],
                    in1=xn[mo][:, n * 512:(n + 1) * 512],
                    op0=mybir.AluOpType.add, op1=mybir.AluOpType.add)
                partials[(mo, n)] = pt
                yield

    qk_res = {}

    def qk_gen(j):
        """Emit pair j's q/k projection (fp8 DR) in chunks."""
        for name, wfun, b_sb in (("q", qwt, qb_sb), ("k", kwt, kb_sb)):
            dst = qk_p.tile([128, L], BF16, tag=f"{name}_j", name=f"{name}_j")
            for n in range(LT):
                acc = sp_p.tile([128, 512], F32, tag="sp", name="qkacc")
                for kc2 in range(KC2):
                    nc.tensor.matmul(
                        acc, wfun(j, kc2),
                        xq[kc2][:, :, n * 512:(n + 1) * 512],
                        start=(kc2 == 0), stop=(kc2 == KC2 - 1),
                        perf_mode=DR)
                    if kc2 % 2 == 1:
                        yield
                nc.vector.tensor_scalar_add(
                    out=dst[:, n * 512:(n + 1) * 512], in0=acc,
                    scalar1=b_sb[:, j:j + 1])
                yield
            qk_res.setdefault(j, []).append(dst)

    for _ in qk_gen(0):
        pass

    with ExitStack() as ph3:
        rc_p = ph3.enter_context(tc.tile_pool(name="rcp", bufs=4))

        ppg = pp_gen()
        for j in range(PAIRS):
            nxt = qk_gen(j + 1) if j + 1 < PAIRS else None
            pdrive = ppg if j >= PAIRS - 2 else None
            q_j, k_j = qk_res.pop(j)

            for tcn in range(LT):
                ps2 = [ps2_p.tile([CH + 1, 512], F32, tag="ps2",
                                  name=f"ps2_{par}") for par in range(2)]
                ex = None
                exs = {}

                def emit_mm2(m):
                    exr = exs[m].rearrange("p (s c) f -> p c s f", c=2)
                    vtr = vT2[m].rearrange("p i (h c) -> p i h c",
                                           c=CH + 1)
                    for par in range(2):
                        h = 2 * j + par
                        nc.tensor.matmul(
                            ps2[par],
                            vtr[:, :, h, :],
                            exr[:, par, :, :],
                            start=(m == 0), stop=(m == KC2 - 1),
                            perf_mode=DR)

                for sc in range(NT):
                    m1 = m1_p.tile([128, 2, 512], F32, tag="m1", name="m1")
                    for par in range(2):
                        base = CH * par
                        nc.tensor.matmul(
                            m1[:, par, :],
                            k_j[base:base + CH, sc * 128:(sc + 1) * 128],
                            q_j[base:base + CH, tcn * 512:(tcn + 1) * 512])
                    sch = sc % 2
                    if sch == 0:
                        ex = ex_p.tile([128, 4, 512], FP8, tag="ex",
                                       name="ex")
                        exs[sc // 2] = ex
                    nc.scalar.activation(
                        out=ex[:, 2 * sch:2 * sch + 2, :], in_=m1,
                        func=mybir.ActivationFunctionType.Exp,
                        bias=ebias, scale=0.125)
                    if nxt is not None:
                        next(nxt, None)
                    if pdrive is not None and sch == 1:
                        next(pdrive, None)
                    # mm2(m) is emitted only after mm1(sc=2m+3): by the
                    # time the in-order PE queue reaches it, exp(m) has
                    # long completed, so the PE never stalls mid-stream
                    # (stalls keep the PE at its mid p-state).
                    if sc >= 3 and sc % 2 == 1 and sc < NT - 1:
                        emit_mm2((sc - 3) // 2)
                # cover exp(6)/exp(7) latency with generator work before
                # the two trailing mm2s so the in-order PE never stalls
                for gen in (nxt, pdrive, nxt):
                    if gen is not None:
                        next(gen, None)
                emit_mm2(KC2 - 2)
                for gen in (nxt, pdrive):
                    if gen is not None:
                        next(gen, None)
                emit_mm2(KC2 - 1)
                # epilogue per head: S row to SBUF (recip must not read
                # PSUM: misreads on HW), reciprocal, broadcast, then
                # multiply the raw PSUM rows directly into the fp8 a-tile.
                for par in range(2):
                    s_sb = rc_p.tile([1, 512], F32, tag="ssb", name="s_sb")
                    nc.vector.tensor_copy(out=s_sb,
                                          in_=ps2[par][CH:CH + 1, :])
                    rc = rc_p.tile([1, 512], F32, tag="rc", name="rc")
                    nc.vector.reciprocal_approx_fast(out=rc, in_=s_sb)
                    sbb = rc_p.tile([CH, 512], F32, tag="sbb", name="sbb")
                    nc.gpsimd.partition_broadcast(sbb, rc, channels=CH)
                    nc.vector.tensor_tensor(
                        out=abuf[j // 2][CH * par:CH * par + CH, j % 2,
                                         tcn * 512:(tcn + 1) * 512],
                        in0=ps2[par][0:CH, :], in1=sbb,
                        op=mybir.AluOpType.mult)
            if nxt is not None:
                for _ in nxt:
                    pass
        for _ in ppg:
            pass

    # ========== Phase 4: last proj step + partial + DMA out ===========
    with ExitStack() as ph4:
        out_p = ph4.enter_context(tc.tile_pool(name="outp", bufs=4))
        for mo in range(NT):
            for n in range(LT):
                acc = sp_p.tile([128, 512], F32, tag="sp", name="pacc")
                nc.tensor.matmul(
                    acc, pwt(mo, KC2 - 1),
                    abuf[KC2 - 1][:, :, n * 512:(n + 1) * 512],
                    start=True, stop=True, perf_mode=DR)
                o_sb = out_p.tile([128, 512], F32, tag="o_sb", name="o_sb")
                nc.vector.tensor_tensor(
                    out=o_sb, in0=acc, in1=partials[(mo, n)],
                    op=mybir.AluOpType.add)
                deng = nc.sync if (mo * LT + n) % 2 == 0 else nc.gpsimd
                deng.dma_start(
                    out=out_ap[mo * 128:(mo + 1) * 128,
                               n * 512:(n + 1) * 512],
                    in_=o_sb)


_CACHED = {}


def build_program(repeats=1):
    key = ("nc", repeats)
    if key in _CACHED:
        return _CACHED[key]
    from contextlib import ExitStack

    nc = bacc.Bacc("TRN2", target_bir_lowering=False, debug=False)
    with tile.TileContext(nc) as tc:
        params = declare_params(nc)
        for rep in range(repeats):
            out_h = None
            if rep > 0:
                out_h = nc.dram_tensor(f"out_scratch{rep}", [C, L], F32)
            with ExitStack() as ctx:
                emit(nc, tc, ctx, params, out_h)
    nc.compile()
    _CACHED[key] = nc
    return nc


def host_pack(norm_w, norm_b, qkv_w, qkv_b, proj_w, proj_b):
    """Precompute packed weight layouts (plain numpy)."""
    f = np.float32
    qkv_w = np.asarray(qkv_w, f)
    qkv_b = np.asarray(qkv_b, f)
    proj_w = np.asarray(proj_w, f)
    proj_b = np.asarray(proj_b, f)

    # pair-packed output index maps (tile j: head 2j at 0:64, 2j+1 at 64:128)
    idx_q = np.empty(C, np.int64)
    idx_k = np.empty(C, np.int64)
    for j in range(PAIRS):
        for p in range(128):
            h = 2 * j + p // CH
            i = p % CH
            idx_q[j * 128 + p] = 192 * h + i
            idx_k[j * 128 + p] = 192 * h + CH + i
    idx_v = np.empty(C, np.int64)
    for h in range(H):
        idx_v[CH * h:CH * (h + 1)] = 192 * h + 2 * CH + np.arange(CH)

    # DoubleRow lhsT packs: tile (o, kc2)[p_c, i_c, col] =
    #   w[row_idx[o, col], 256*kc2 + 128*i_c + p_c]
    def pack_dr(w, row_idx, n_out_tiles, out_w):
        nt = n_out_tiles
        res = np.empty((nt, KC2, 128, 2, out_w), f)
        for o in range(nt):
            rows = w[row_idx[o]]  # [out_w, C]
            blk = rows.reshape(out_w, KC2, 2, 128)  # c = 256*kc2+128*i+p
            res[o] = blk.transpose(1, 3, 2, 0)  # [KC2, 128, 2, out_w]
        # partition-major: [128, (o, kc2, 2, out_w)] contiguous per row
        return np.ascontiguousarray(
            res.transpose(2, 0, 1, 3, 4).reshape(128, nt * KC2 * 2 * out_w)
        ).astype(NPFP8)

    q_w8 = pack_dr(qkv_w, idx_q.reshape(PAIRS, 128), PAIRS, 128)
    k_w8 = pack_dr(qkv_w, idx_k.reshape(PAIRS, 128), PAIRS, 128)
    v_w8 = pack_dr(qkv_w, idx_v.reshape(2, 512), 2, 512)
    p_w8 = pack_dr(proj_w, np.arange(C).reshape(NT, 128), NT, 128)

    q_b = np.ascontiguousarray(qkv_b[idx_q].reshape(NT, 128).T)
    k_b = np.ascontiguousarray(qkv_b[idx_k].reshape(NT, 128).T)
    pbe = proj_b + proj_w @ qkv_b[idx_v]
    proj_beff = np.ascontiguousarray(pbe.astype(f).reshape(NT, 128).T)

    norm_w_c = np.ascontiguousarray(np.asarray(norm_w, f).reshape(NT, 128).T)
    norm_b_c = np.ascontiguousarray(np.asarray(norm_b, f).reshape(NT, 128).T)

    pp = np.arange(128)
    A_grp = (pp[:, None] // 32 == np.arange(4)[None, :]).astype(f)
    A2T = np.ascontiguousarray(A_grp.T)

    return dict(
        q_w8=q_w8, k_w8=k_w8, v_w8=v_w8, p_w8=p_w8,
        q_b=q_b, k_b=k_b, proj_beff=proj_beff,
        norm_w_c=norm_w_c, norm_b_c=norm_b_c, A_grp=A_grp, A2T=A2T,
    )


def kernel(x, norm_w, norm_b, qkv_w, qkv_b, proj_w, proj_b, _trace=False):
    x = np.asarray(x, np.float32)
    shared = host_pack(norm_w, norm_b, qkv_w, qkv_b, proj_w, proj_b)
    nc = build_program()
    in_maps = [dict(shared, x=np.ascontiguousarray(x[i])) for i in range(B)]
    res = run_bass_kernel_spmd(nc, in_maps, list(range(B)), trace=_trace)
    out = np.stack([res.results[i]["out"] for i in range(B)], axis=0)
    if _trace:
        kernel._last_results = res
    return out.astype(np.float32)
